# revision 22
# baseline (speedup 1.0000x reference)
"""Trainium2 Bass kernel for the atomic-descriptor builder (radial Chebyshev +
angular Legendre descriptors, N=256 atoms, minimum-image PBC).

Strategy: shard the central-atom axis i across 8 NeuronCores (32 atoms each).
Per core, pairs live as [128 j-partitions, 2 j-chunks x 32 atoms free].
The O(N^3) triplet sum is reformulated exactly via the monomial expansion of
the Legendre polynomials into symmetric tensor powers of the unit bond
vectors u_ij:

  q_ang[i,n,l] = sum_c A[c,l] * M[i,n,c]^2,
  M[i,n,c] = sum_j g[i,j,n] * (u_ij)^{c}        (35 monomials, deg<=4)

Key device tricks:
 - 41-row fp16 component tile: 35 monomials + 6 scratch rows that cyclically
   extend (x,y | xx,yy | xy,yz), so every power is one wide [128,3,W] fp16
   product; permutation weights live in the A matrix.
 - Radial features are emitted in a mixed basis: true Chebyshev features
   0..3 (all the angular part needs) plus raw powers x^4..x^8 * tcos, which
   need only a shallow product tree instead of the full T4..T8 ladder; the
   host multiplies the per-atom radial row by an exact constant 9x9 basis
   matrix afterwards.
 - PBC wrap via round-to-int on gpsimd (f32->i32 conversion rounds to
   nearest even, exactly matching jnp.round).
 - One act-table pays for Sqrt+Square; the Sin load for the cosine cutoff
   happens mid-kernel while ACT is otherwise idle.
 - fp16 PE matmuls (4x faster per moving row than fp32).
 - I/O through SWDGE prepare+trigger: the input arrives via an indexed
   gather fired at t~1us, outputs leave via kv_writeback (q_ang) and a
   scatter-add into a pre-zeroed strip (q_r), cutting the DMA fixed costs
   (desc-gen + DGE delay) out of the critical path."""
import numpy as np

N_ATOMS = 256
NCORES = 8
NI = N_ATOMS // NCORES        # 32 central atoms per core
NCHUNK = 2                    # j-chunks of 128 partitions
W = NCHUNK * NI               # 64 free columns per (chunk, atom)
NFEAT = 9                     # radial features (K_RADIAL+1)
NA = 4                        # angular radial features
RC = 5.0
NROW = 41                     # component rows incl. 6 scratch
GRP = 8                       # atoms per PSUM bank
# fused input block columns: si_rep | sj | mask | A | pad
C_SI, C_SJ, C_MASK, C_A = 0, 3 * W, 3 * W + 6, 3 * W + 6 + W
NCOL = 320                    # padded so gather rows are 1280B (256B mult)

# rows: 0:'1' 1:x 2:y 3:z [4,5]:x,y 6:xx 7:yy 8:zz [9,10]:xx,yy
# 11:xy 12:yz 13:xz [14,15]:xy,yz 16-18:D*u 19-21:D*rot1(u)
# 22-24:D*rot2(u) 25:xyz 26-28:D*D 29-31:D*rot1(D) 32-34:D*R
# 35-37:D*rot2(R) 38-40:D*rot1(R)
DEG = [0, 1, 1, 1, -1, -1, 2, 2, 2, -1, -1, 2, 2, 2, -1, -1,
       3, 3, 3, 3, 3, 3, 3, 3, 3, 3, 4, 4, 4, 4, 4, 4, 4, 4, 4,
       4, 4, 4, 4, 4, 4]
WGT = [1, 1, 1, 1, 0, 0, 1, 1, 1, 0, 0, 2, 2, 2, 0, 0,
       1, 1, 1, 3, 3, 3, 3, 3, 3, 6, 1, 1, 1, 6, 6, 6, 4, 4, 4,
       4, 4, 4, 12, 12, 12]
# Legendre-in-monomial coefficients: q_l = sum_p CLP[l][p] * S_p
CLP = np.array([
    [1.0, 0, 0, 0, 0],
    [0, 1.0, 0, 0, 0],
    [-0.5, 0, 1.5, 0, 0],
    [0, -1.5, 0, 2.5, 0],
    [0.375, 0, -3.75, 0, 4.375],
], dtype=np.float64)


def _amat():
    """[41, 5] fp32: A[c,l] = w_c * CLP[l, deg_c]; scratch rows zero."""
    A = np.zeros((NROW, 5), np.float64)
    for c in range(NROW):
        if DEG[c] >= 0:
            A[c] = WGT[c] * CLP[:, DEG[c]]
    return A.astype(np.float32)


def _qr_basis():
    """Exact 9x9 map device-radial-features -> true Chebyshev features.

    Device features over monomials m_p = 0.5*x^p*fc*mask:
      f0=2m0, f1=m0+m1, f2=2m2, f3=m0-3m1+4m3, f_{4+j}=m_{4+j}.
    True:  phi_k = m0 + sum_p cheb(T_k)_p m_p  (phi_0 = 2m0)."""
    F = np.zeros((9, 9))
    F[0, 0] = 2.0
    F[1, 0] = 1.0
    F[1, 1] = 1.0
    F[2, 2] = 2.0
    F[3, 0], F[3, 1], F[3, 3] = 1.0, -3.0, 4.0
    for j in range(5):
        F[4 + j, 4 + j] = 1.0
    Phi = np.zeros((9, 9))
    Phi[0, 0] = 2.0
    for k in range(1, 9):
        cheb = np.polynomial.chebyshev.Chebyshev.basis(k).convert(
            kind=np.polynomial.Polynomial).coef
        Phi[k, :len(cheb)] += cheb
        Phi[k, 0] += 1.0
    return (Phi @ np.linalg.inv(F)).astype(np.float64)


_compiled = {}


def _build_program(box):
    import concourse.bass as bass
    import concourse.bacc as bacc
    import concourse.tile as tile
    from concourse import mybir

    f32 = mybir.dt.float32
    f16 = mybir.dt.float16
    i16 = mybir.dt.int16
    i32 = mybir.dt.int32
    op = mybir.AluOpType
    act = mybir.ActivationFunctionType
    pi = float(np.pi)

    boxf = np.asarray(box, np.float32)
    L = float(boxf[0, 0])
    diag_box = float(np.abs(boxf - np.diag(np.diag(boxf))).max()) == 0.0
    eq_diag = diag_box and boxf[0, 0] == boxf[1, 1] == boxf[2, 2]
    assert eq_diag, "kernel specialised to cubic boxes (reference uses one)"

    nc = bacc.Bacc("TRN2", target_bir_lowering=False, debug=False,
                   enable_asserts=False)

    insd = nc.dram_tensor("ins", [128, NCOL], f32, kind="ExternalInput")
    oqrd = nc.dram_tensor("oqr", [1, 320], f32, kind="ExternalOutput")
    oangd = nc.dram_tensor("oang", [1, 128, 1, 5], f32, kind="ExternalOutput")

    with tile.TileContext(nc) as tc:
        with tc.tile_pool(name="sb", bufs=1) as sb, \
             tc.tile_pool(name="ps", bufs=1, space="PSUM") as ps:

            def t(shape, tag, dt=f32):
                return sb.tile(shape, dt, tag=tag, name=tag)

            V, P, S = nc.vector, nc.gpsimd, nc.scalar

            # ---- t~0: constants, DMA preps, input gather --------------
            eps_b = t([128, 1], "eps_b")
            V.memset(eps_b, 1e-12)
            m1_b = t([128, 1], "m1_b")
            V.memset(m1_b, -1.0)
            hp_b = t([128, 1], "hp_b")
            V.memset(hp_b, pi / 2)
            qrp = t([128, 320], "qrp")        # scatter payload, row 0 only
            V.memset(qrp[0:1, :], 0.0)
            z5 = t([5, 64], "z5")
            V.memset(z5[:, :], 0.0)
            idx_in = t([16, 8], "idx_in", i16)
            P.iota(idx_in[:, :], pattern=[[16, 8]], base=0,
                   channel_multiplier=1)
            idx_qr = t([16, 1], "idx_qr", i16)
            P.memset(idx_qr[:, :], 0)
            ctx0 = t([128, 1], "ctx0", i32)
            P.memset(ctx0[:, :], 0)

            ins = t([128, NCOL], "ins")
            nc.sync.dma_start(out=ins[:, :], in_=insd.ap())
            # pre-zero the q_r scatter strip while compute runs
            nc.sync.dma_start(
                out=bass.AP(tensor=oqrd.ap().tensor, offset=0,
                            ap=[[64, 5], [1, 64]]),
                in_=z5[:, :])

            mask = ins[:, C_MASK:C_MASK + W]
            Af = ins[0:NROW, C_A:C_A + 5]
            A16 = t([NROW, 5], "A16", f16)
            V.tensor_scalar(out=A16[:, :], in0=Af, scalar1=1.0,
                            scalar2=None, op0=op.mult)

            Tt = t([128, NROW, W], "Tt", f16)
            V.memset(Tt[:, 0, :], 1.0)

            def bc(ap_sl, n):
                # broadcast [128, w] -> [128, n, w] via stride-0 middle dim
                return bass.AP(tensor=ap_sl.tensor, offset=ap_sl.offset,
                               ap=[ap_sl.ap[0], [0, n], ap_sl.ap[1]])

            # ---- minimum-image displacements (fractional) -------------
            ds = t([128, 3, W], "ds")
            rnd = t([128, 3, W], "rnd", i32)
            si = ins[:, C_SI:C_SI + 3 * W].rearrange("p (d w) -> p d w", d=3)
            for c in range(NCHUNK):
                cs = slice(c * NI, (c + 1) * NI)
                sj = ins[:, C_SJ + 3 * c:C_SJ + 3 * (c + 1)]
                sj_b = bass.AP(tensor=sj.tensor, offset=sj.offset,
                               ap=[sj.ap[0], sj.ap[1], [0, NI]])
                V.tensor_tensor(out=ds[:, :, cs], in0=si[:, :, cs],
                                in1=sj_b, op=op.subtract)
                P.tensor_scalar(out=rnd[:, :, cs], in0=ds[:, :, cs],
                                scalar1=1.0, scalar2=None, op0=op.mult)
                V.tensor_tensor(out=ds[:, :, cs], in0=ds[:, :, cs],
                                in1=rnd[:, :, cs], op=op.subtract)

            # ---- pair distances -------------------------------------
            dr2 = t([128, 3, W], "dr2")
            rsq = t([128, W], "rsq")
            for c in range(NCHUNK):
                cs = slice(c * NI, (c + 1) * NI)
                P.tensor_tensor(out=dr2[:, :, cs], in0=ds[:, :, cs],
                                in1=ds[:, :, cs], op=op.mult)
                V.tensor_reduce(
                    out=rsq[:, cs],
                    in_=dr2[:, :, cs].rearrange("p d w -> p w d"),
                    axis=mybir.AxisListType.X, op=op.add)
            rij = t([128, W], "rij")          # fractional
            S.activation(out=rij[:, :], in_=rsq[:, :], func=act.Sqrt,
                         bias=eps_b[:, :])
            rinv = t([128, W], "rinv")
            V.reciprocal(out=rinv[:, :], in_=rij[:, :])
            t2 = t([128, W], "t2")            # (r/RC - 1)^2
            S.activation(out=t2[:, :], in_=rij[:, :], func=act.Square,
                         scale=L / RC, bias=m1_b[:, :])
            xcl = t([128, W], "xcl")          # min(r, RC)/L for the Sin
            V.tensor_scalar(out=xcl[:, :], in0=rij[:, :], scalar1=RC / L,
                            scalar2=None, op0=op.min)
            cosv = t([128, W], "cosv")        # cos(pi*min(r,RC)/RC)
            S.activation(out=cosv[:, :], in_=xcl[:, :], func=act.Sin,
                         scale=-pi * L / RC, bias=hp_b[:, :])

            # ---- tensor powers of unit vectors (fp16) -----------------
            u = Tt[:, 1:4, :]
            D = Tt[:, 6:9, :]
            R = Tt[:, 11:14, :]
            V.tensor_tensor(out=u, in0=ds[:, :, :], in1=bc(rinv[:, :], 3),
                            op=op.mult)
            V.tensor_scalar(out=Tt[:, 4:6, :], in0=Tt[:, 1:3, :], scalar1=1.0,
                            scalar2=None, op0=op.mult)      # ext x,y
            V.tensor_tensor(out=D, in0=u, in1=u, op=op.mult)
            V.tensor_tensor(out=R, in0=u, in1=Tt[:, 2:5, :], op=op.mult)
            V.tensor_scalar(out=Tt[:, 9:11, :], in0=Tt[:, 6:8, :],
                            scalar1=1.0, scalar2=None, op0=op.mult)
            V.tensor_scalar(out=Tt[:, 14:16, :], in0=Tt[:, 11:13, :],
                            scalar1=1.0, scalar2=None, op0=op.mult)

            # ---- Chebyshev T1-T3 + raw powers x^3..x^8 ----------------
            Tsm = t([128, 3, W], "Tsm")       # rows x=T1, T2, T3
            XP = t([128, 5, W], "XP")         # rows x^4..x^8
            x = Tsm[:, 0, :]
            x2 = t([128, W], "x2")
            x3 = t([128, W], "x3")
            t3m = t([128, W], "t3m")
            V.tensor_scalar(out=x, in0=t2[:, :], scalar1=2.0, scalar2=-1.0,
                            op0=op.mult, op1=op.add)
            V.tensor_tensor(out=x2[:, :], in0=x, in1=x, op=op.mult)
            P.tensor_scalar(out=Tsm[:, 1, :], in0=x2[:, :], scalar1=2.0,
                            scalar2=-1.0, op0=op.mult, op1=op.add)
            V.tensor_tensor(out=x3[:, :], in0=x, in1=x2[:, :], op=op.mult)
            V.tensor_tensor(out=t3m[:, :], in0=x, in1=Tsm[:, 1, :],
                            op=op.mult)
            V.scalar_tensor_tensor(out=Tsm[:, 2, :], in0=t3m[:, :],
                                   scalar=2.0, in1=x, op0=op.mult,
                                   op1=op.subtract)
            P.tensor_tensor(out=XP[:, 0, :], in0=x2[:, :], in1=x2[:, :],
                            op=op.mult)
            P.tensor_tensor(out=XP[:, 1, :], in0=x2[:, :], in1=x3[:, :],
                            op=op.mult)
            P.tensor_tensor(out=XP[:, 2, :], in0=x3[:, :], in1=x3[:, :],
                            op=op.mult)
            P.tensor_tensor(out=XP[:, 3, :], in0=x3[:, :], in1=XP[:, 0, :],
                            op=op.mult)
            P.tensor_tensor(out=XP[:, 4, :], in0=XP[:, 0, :],
                            in1=XP[:, 0, :], op=op.mult)

            # remaining tensor powers on DVE, independent of the x-chain
            V.tensor_tensor(out=Tt[:, 16:19, :], in0=D, in1=u, op=op.mult)
            V.tensor_tensor(out=Tt[:, 19:22, :], in0=D, in1=Tt[:, 2:5, :],
                            op=op.mult)
            V.tensor_tensor(out=Tt[:, 22:25, :], in0=D, in1=Tt[:, 3:6, :],
                            op=op.mult)
            V.tensor_tensor(out=Tt[:, 25, :], in0=Tt[:, 11, :],
                            in1=Tt[:, 3, :], op=op.mult)
            V.tensor_tensor(out=Tt[:, 26:29, :], in0=D, in1=D, op=op.mult)
            V.tensor_tensor(out=Tt[:, 29:32, :], in0=D, in1=Tt[:, 7:10, :],
                            op=op.mult)
            V.tensor_tensor(out=Tt[:, 32:35, :], in0=D, in1=R, op=op.mult)
            V.tensor_tensor(out=Tt[:, 35:38, :], in0=D, in1=Tt[:, 13:16, :],
                            op=op.mult)
            V.tensor_tensor(out=Tt[:, 38:41, :], in0=D, in1=Tt[:, 12:15, :],
                            op=op.mult)

            # ---- cutoff & radial features (fp16) ----------------------
            maskc = t([128, W], "maskc")      # (r<rc) * 0.25-scaled mask
            V.scalar_tensor_tensor(out=maskc[:, :], in0=rij[:, :],
                                   scalar=RC / L, in1=mask,
                                   op0=op.is_lt, op1=op.mult)
            tcos = t([128, W], "tcos")        # 0.5 * fc * mask
            V.scalar_tensor_tensor(out=tcos[:, :], in0=cosv[:, :], scalar=1.0,
                                   in1=maskc[:, :], op0=op.add, op1=op.mult)
            phi = t([128, NFEAT, W], "phi", f16)
            P.tensor_scalar(out=phi[:, 0, :], in0=tcos[:, :], scalar1=2.0,
                            scalar2=None, op0=op.mult)
            for c in range(NCHUNK):
                cs = slice(c * NI, (c + 1) * NI)
                V.scalar_tensor_tensor(           # (T_k+1)*tcos, k=1..3
                    out=phi[:, 1:4, cs], in0=Tsm[:, :, cs], scalar=1.0,
                    in1=bc(tcos[:, cs], 3), op0=op.add, op1=op.mult)
                eng = V if c == 0 else P
                eng.tensor_tensor(                # x^p * tcos, p=4..8
                    out=phi[:, 4:9, cs], in0=XP[:, :, cs],
                    in1=bc(tcos[:, cs], 5), op=op.mult)

            # ---- per-atom reductions over j (PE matmuls, fp16) --------
            pm = [ps.tile([NROW, GRP, NFEAT], f32, tag=f"pm{w}",
                          name=f"pm{w}") for w in range(NI // GRP)]
            for i in range(NI):
                wv, il = divmod(i, GRP)
                for c in range(NCHUNK):
                    col = c * NI + i
                    nc.tensor.matmul(pm[wv][:, il, :],
                                     Tt[:, :, col:col + 1],
                                     phi[:, :, col:col + 1],
                                     start=(c == 0), stop=(c == NCHUNK - 1))

            qang = t([128, 5], "qang")

            # ---- squares -> Legendre fold -> outputs ------------------
            M2 = t([NROW, NI, NA], "M2", f16)
            qang_ps = ps.tile([128, 5], f32, tag="qang_ps", name="qang_ps")
            for wv in range(NI // GRP):
                lo, hi = wv * GRP, (wv + 1) * GRP
                S.activation(out=M2[:, lo:hi, :], in_=pm[wv][:, :, 0:NA],
                             func=act.Square)
                V.tensor_scalar(out=qrp[0:1, 72 * wv:72 * (wv + 1)],
                                in0=pm[wv][0:1, :, :].rearrange(
                                    "p a f -> p (a f)"),
                                scalar1=1.0, scalar2=None, op0=op.mult)
                if wv % 2 == 1:
                    l2 = (wv - 1) * GRP
                    nc.tensor.matmul(qang_ps[4 * l2:4 * hi, :],
                                     M2[:, l2:hi, :], A16[:, :],
                                     start=True, stop=True)
                    V.tensor_scalar(out=qang[4 * l2:4 * hi, :],
                                    in0=qang_ps[4 * l2:4 * hi, :],
                                    scalar1=1.0, scalar2=None, op0=op.mult)
            nc.sync.dma_start(
                out=bass.AP(tensor=oqrd.ap().tensor, offset=0,
                            ap=[[0, 1], [1, 320]]),
                in_=qrp[0:1, :])
            # qang leaves on ACT's HWDGE queue so both output descriptor
            # generations overlap instead of serialising on SP
            nc.scalar.dma_start(out=oangd.ap().opt([0, 2]),
                                in_=qang[:, :])

    nc.compile()
    return nc


def _host_prep(R, box):
    R = np.asarray(R, np.float32)
    box = np.asarray(box, np.float32)
    box_inv = np.linalg.inv(box)
    s = np.ascontiguousarray((R @ box_inv.T).astype(np.float32))
    A = _amat()
    in_maps = []
    for r in range(NCORES):
        sl = s[r * NI:(r + 1) * NI, :]                    # [NI, 3]
        ins = np.zeros((128, NCOL), np.float32)
        for c in range(NCHUNK):
            for d in range(3):
                ins[:, d * W + c * NI:d * W + (c + 1) * NI] = sl[:, d]
            ins[:, C_SJ + 3 * c:C_SJ + 3 * (c + 1)] = \
                s[c * 128:(c + 1) * 128, :]
        mask = np.full((128, W), 0.25, np.float32)
        for i in range(NI):
            g = r * NI + i
            c, j = divmod(g, 128)
            mask[j, c * NI + i] = 0.0
        ins[:, C_MASK:C_MASK + W] = mask
        ins[0:NROW, C_A:C_A + 5] = A
        in_maps.append({"ins": ins})
    return in_maps


def kernel(R, box):
    R = np.asarray(R)
    box = np.asarray(box)
    key = np.asarray(box, np.float32).tobytes()
    nc = _compiled.get(key)
    if nc is None:
        nc = _build_program(box)
        _compiled[key] = nc
    in_maps = _host_prep(R, box)
    from concourse.bass_utils import run_bass_kernel_spmd
    res = run_bass_kernel_spmd(nc, in_maps, core_ids=list(range(NCORES)))
    B = _qr_basis()
    parts = []
    for r in range(NCORES):
        qr_dev = res.results[r]["oqr"].reshape(320)[:NI * 9].reshape(NI, 9)
        qr = (qr_dev.astype(np.float64) @ B.T).astype(np.float32)
        qa = res.results[r]["oang"].reshape(NI, NA * 5)  # rows i*4+n, col l
        parts.append(np.concatenate([qr, qa], axis=1))
    return np.concatenate(parts, axis=0).astype(np.float32)
